# revision 21
# baseline (speedup 1.0000x reference)
"""MoE layer (dense experts) on 8 Trainium2 NeuronCores via Bass/Tile.

Problem (hardcoded shapes):
  x        [4, 2048, 1024] f32
  gate_w   [1024, 8] f32, gate_b [8] f32
  expert_w [8, 1024, 1024] f32, expert_b [8, 1024] f32
  out[b,t,p] = sum_e softmax(x @ gate_w + gate_b)[b,t,e]
               * (x @ expert_w[e] + expert_b[e])[b,t,p]

Sharding: data-parallel over tokens. 8192 tokens are split into 8 shards of
1024; every core gets the full gate/expert weights (replicated) and computes
its token shard end-to-end. No collectives.

Per-core kernel (x pre-transposed on host so the contraction dim is the
partition dim for both matmul operands):
  - gating logits computed TRANSPOSED: lgT[e, t] accumulates
    gw_d[128,8].T @ xT_d[128,512] over 8 d-tiles. exp via ACT, gates
    transposed to [tok, E] with DVE 32x32 stream transposes and
    normalized per 128-token tile.
  - expert e: psum[t128, p512] accumulates sum_d xT[d,t].T @ w_e[d,p] over
    8 d-tiles.
  - phase A is d-outer over (lgT0, ti0-3 pc0, lgT1, ti4-5 pc0): per-d-step
    HBM demand is xt tile (256KB) + w0 pc0 half (128KB) ~ 220GB/s, under
    the 358GB/s per-core HBM cap (interleaving both w0 halves would need
    ~300GB/s and stalls the PE during the ramp).
  - gate-weighted sum on DVE: acc = psum_e * g[:,e] + acc (one fused
    scalar_tensor_tensor per psum tile); the last expert's stt writes a
    per-ti bf16 staging tile, stored as one [128,1024] DMA per ti.
  - DMA count is kept low (~50 vs ~100): the end-of-program teardown and
    per-trigger engine cost (~700ns each) scale with it. Expert weights
    for e>=1 stream as 2x 1MB DMAs per expert, round-robin over the
    Sync/Scalar HWDGE + GpSimd SWDGE queues, issued one expert ahead.
  - PE warm-up: ~8 dummy matmuls over a memset tile bridge the HAM clock
    gate (PE at 1.2GHz until ~3.4us of sustained activity) while the
    first DMAs land.
  - biases: the harness inputs have gate_b = expert_b = 0, checked at
    runtime; the specialized no-bias module skips the bias-mix matmuls.
Matmul dtype: bf16 (default) or float32r/fp32 via MOE_MM_DTYPE.
"""

import os
from contextlib import ExitStack

import numpy as np

import concourse.bacc as bacc
import concourse.bass as bass
import concourse.mybir as mybir
import concourse.tile as tile
from concourse.bass_utils import run_bass_kernel_spmd

B, T, D, E, P = 4, 2048, 1024, 8, 1024
N_CORES = 8
TOK = B * T                # 8192 tokens
TS = TOK // N_CORES        # 1024 tokens per core
DT = D // 128              # 8 contraction tiles
TT = TS // 128             # 8 token tiles per core
PCHUNK = 512               # psum bank free size (f32)
PC = P // PCHUNK           # 2 p-chunks
TA = 6                     # token tiles in phase-A (pc0) d-outer phase
                           # (TA + 2 logit banks = 8 psum banks)
TCH = TS // PCHUNK         # 2 token chunks for the gating matmul
HALF = TS // 2             # xT d-tile DMA half (per-queue split)

_F32 = mybir.dt.float32
_BF16 = mybir.dt.bfloat16

MM_DTYPE = os.environ.get("MOE_MM_DTYPE", "bf16")
TRACE = os.environ.get("MOE_TRACE", "0") == "1"  # test.py sets this for profiling

_mm_dt = {
    "fp32r": mybir.dt.float32r,
    "bf16": mybir.dt.bfloat16,
    "fp32": mybir.dt.float32,
}

_build_cache = {}


def _build(mode: str, with_bias: bool) -> bass.Bass:
    mm = _mm_dt[mode]
    nc = bacc.Bacc("TRN2", target_bir_lowering=False, debug=False,
                   num_devices=N_CORES)

    xT = nc.dram_tensor("xT", [D, TS], mm, kind="ExternalInput").ap()
    # gate_w pre-arranged on host to [128, DT*E] (dp-major) for one
    # contiguous DMA
    gw = nc.dram_tensor("gate_w", [128, DT * E], mm, kind="ExternalInput").ap()
    ew = nc.dram_tensor("expert_w", [E, D, P], mm, kind="ExternalInput").ap()
    if with_bias:
        ident = nc.dram_tensor("ident", [128, 128], _F32,
                               kind="ExternalInput").ap()
        gb = nc.dram_tensor("gate_b", [E, 1], _F32, kind="ExternalInput").ap()
        eb = nc.dram_tensor("expert_b", [E, P], _BF16, kind="ExternalInput").ap()
    out = nc.dram_tensor("out", [TS, P], _BF16, kind="ExternalOutput").ap()

    out_t = out.rearrange("(tt tp) p -> tp tt p", tp=128)
    xT_t = xT.rearrange("(dt dp) t -> dp dt t", dp=128)
    # expert_w as [dp, e, dt, p] so a [128, ndt, P] slab is one DMA
    ew_t = ew.rearrange("e (dt dp) p -> dp e dt p", dp=128)

    with tile.TileContext(nc) as tc, ExitStack() as ctx:
        consts = ctx.enter_context(tc.tile_pool(name="consts", bufs=1))
        w0_pool = ctx.enter_context(tc.tile_pool(name="w0", bufs=1))
        we_pool = ctx.enter_context(tc.tile_pool(name="we", bufs=3))
        stage_pool = ctx.enter_context(tc.tile_pool(name="stage", bufs=3))
        stats = ctx.enter_context(tc.tile_pool(name="stats", bufs=4))
        psum = ctx.enter_context(tc.tile_pool(name="psum", bufs=8, space="PSUM"))

        # PE warm-up: the HAM clock gate keeps the PE at 1.2GHz until it
        # has seen ~3.4us of sustained matmul activity. The first real
        # matmul can't start before ~8us (framework preamble + DMA
        # latency), so burn the wait on dummy matmuls to enter warm.
        warm = consts.tile([128, PCHUNK], mm, name="warm", tag="warm")
        nc.vector.memset(warm[:, :], 0.0)
        wps = psum.tile([128, PCHUNK], _F32, name="warm_ps", tag="ps")
        for i in range(4):
            nc.tensor.matmul(wps[:, :], warm[:, :128], warm[:, :],
                             start=True, stop=True)

        # Ramp: gw first (first matmul needs it), then per-d-tile pieces in
        # phase-A consumption order: xt halves on the two HWDGE queues,
        # w0 pc0 half-tiles on the GpSimd SWDGE queue. The w0 pc1 halves
        # and expert-1 weights follow as merged DMAs.
        # Ramp queue plan. The Scalar queue runs ~1.4us behind Sync (the
        # framework's ACT_TABLE_LOAD precedes its first trigger), and the
        # GpSimd SWDGE path has ~2us completion latency with heavy jitter
        # — so every piece the first two phase-A steps depend on (gw,
        # early xt halves, w0 d0/d1 pc0) goes on the HWDGE queues, with
        # gw first on Sync. Only late-needed w0 tiles ride SWDGE.
        gw_sb = consts.tile([128, DT, E], mm, name="gw_sb", tag="gw_sb")
        nc.sync.dma_start(gw_sb[:, :, :],
                          gw.rearrange("dp (dt e) -> dp dt e", e=E))
        xt = consts.tile([128, DT, TS], mm, name="xt", tag="xt")
        w0 = consts.tile([128, DT, P], mm, name="w0", tag="w0")
        for di in range(DT):
            nc.sync.dma_start(xt[:, di, :HALF], xT_t[:, di, :HALF])
            nc.scalar.dma_start(xt[:, di, HALF:], xT_t[:, di, HALF:])
            if di < 2:
                nc.sync.dma_start(w0[:, di, :PCHUNK],
                                  ew_t[:, 0, di, :PCHUNK])
            elif di < 4:
                nc.scalar.dma_start(w0[:, di, :PCHUNK],
                                    ew_t[:, 0, di, :PCHUNK])
            else:
                nc.gpsimd.dma_start(w0[:, di, :PCHUNK],
                                    ew_t[:, 0, di, :PCHUNK])
        # w0 pc1 halves, merged: [128, 4, 512] each
        nc.gpsimd.dma_start(w0[:, 0:4, PCHUNK:], ew_t[:, 0, 0:4, PCHUNK:])
        nc.gpsimd.dma_start(w0[:, 4:8, PCHUNK:], ew_t[:, 0, 4:8, PCHUNK:])
        if with_bias:
            id_sb = consts.tile([128, 128], _F32, name="id_sb", tag="id_sb")
            nc.sync.dma_start(id_sb[:, :], ident)
            gb_sb = consts.tile([E, 1], _F32, name="gb_sb", tag="gb_sb")
            nc.sync.dma_start(gb_sb[:, :], gb)
            eb_sb = consts.tile([E, P], _BF16, name="eb_sb", tag="eb_sb")
            nc.sync.dma_start(eb_sb[:, :], eb)

        gn_sb = consts.tile([128, TT, E], _F32, name="gn_sb", tag="gn_sb")  # normalized
        if with_bias:
            gexpT = consts.tile([E, TS], _F32, name="gexpT", tag="gexpT")
            g_sb = consts.tile([128, TT, E], _F32, name="g_sb", tag="g_sb")
        else:
            # gexpT padded to 32 partitions so the DVE 32x32 stream
            # transpose can produce the [tok, E] gate layout without
            # spending tensor-engine time; rows E..31 stay zero
            gexpT = consts.tile([32, TS], _F32, name="gexpT", tag="gexpT")
            nc.vector.memset(gexpT[:, :], 0.0)
            g_sb = consts.tile([128, TT, 32], _F32, name="g_sb", tag="g_sb")
            junk = consts.tile([E, TCH], _F32, name="junk", tag="junk")
        if with_bias:
            gtn = consts.tile([E, TS], _BF16, name="gtn", tag="gtn")  # normalized gT
        acc = consts.tile([128, TT, P], _F32, name="acc", tag="acc")

        # expert e>=1 weight tiles: one [128, DT, P] slab per expert,
        # filled by two 1MB DMAs; we_pool bufs=3 gives ~2 experts of
        # prefetch lead. Queues split over the two HWDGE rings.
        we_tiles = {}

        _eq = [(nc.sync, nc.scalar), (nc.gpsimd, nc.sync),
               (nc.scalar, nc.gpsimd)]

        def issue_expert_dma(e):
            w_tile = we_pool.tile([128, DT, P], mm, name=f"we{e}", tag="we")
            qa, qb = _eq[(e - 1) % 3]
            qa.dma_start(w_tile[:, 0:4, :], ew_t[:, e, 0:4, :])
            qb.dma_start(w_tile[:, 4:8, :], ew_t[:, e, 4:8, :])
            we_tiles[e] = w_tile

        issue_expert_dma(1)

        # --- phase A: gating logits (transposed) + expert-0 pc0 for token
        # tiles 0..TA-1, d-outer so compute starts as each piece lands.
        lgT = [psum.tile([E, PCHUNK], _F32, name=f"lgT{tch}", tag="ps")
               for tch in range(TCH)]
        ps_grp = {}
        for ti in range(TA):
            ps_grp[ti] = psum.tile([128, PCHUNK], _F32,
                                   name=f"ps0_{ti}_0", tag="ps")
        # Per d-step order: lgT0 + ti0-3 need only the Sync-queue x half
        # (first to arrive: the Scalar queue's x halves run ~1.3us later
        # behind the ACT table load); lgT1 + ti4-5 need the Scalar half.
        # w0 demand is the pc0 half only (full-w0 interleave would need
        # ~300GB/s and stall the ramp).
        for di in range(DT):
            nc.tensor.matmul(lgT[0][:, :], gw_sb[:, di, :],
                             xt[:, di, 0:PCHUNK],
                             start=(di == 0), stop=(di == DT - 1))
            for ti in range(4):
                nc.tensor.matmul(
                    ps_grp[ti][:, :],
                    xt[:, di, ti * 128:(ti + 1) * 128],
                    w0[:, di, 0:PCHUNK],
                    start=(di == 0), stop=(di == DT - 1))
            nc.tensor.matmul(lgT[1][:, :], gw_sb[:, di, :],
                             xt[:, di, PCHUNK:2 * PCHUNK],
                             start=(di == 0), stop=(di == DT - 1))
            for ti in range(4, TA):
                nc.tensor.matmul(
                    ps_grp[ti][:, :],
                    xt[:, di, ti * 128:(ti + 1) * 128],
                    w0[:, di, 0:PCHUNK],
                    start=(di == 0), stop=(di == DT - 1))

        # --- gating epilogue: exp (no max-sub), DVE transpose per token
        # tile, normalize in [tok, E] layout.
        for tch in range(TCH):
            sl = slice(tch * PCHUNK, (tch + 1) * PCHUNK)
            nc.scalar.activation(gexpT[:E, sl], lgT[tch][:, :],
                                 mybir.ActivationFunctionType.Exp,
                                 bias=gb_sb[:, :] if with_bias else 0.0,
                                 scale=1.0)
            if not with_bias:
                # ordering crutch: a DVE read of the exp output ahead of
                # the stream transposes in the DVE queue guarantees the
                # ACT->DVE dependency even if InstStreamTranspose inputs
                # aren't tracked across engines
                nc.vector.tensor_copy(junk[:, tch:tch + 1],
                                      gexpT[:E, tch * PCHUNK:
                                            tch * PCHUNK + 1])

        def gate_tile(ti):
            tsl = slice(ti * 128, (ti + 1) * 128)
            if with_bias:
                tp = psum.tile([128, E], _F32, name=f"tp{ti}", tag="ps")
                nc.tensor.transpose(tp[:, :], gexpT[:, tsl], id_sb[:E, :E])
                nc.vector.tensor_copy(g_sb[:, ti, :], tp[:, :])
            else:
                # [32-pad, 128] -> [128, 32-pad] via four DVE 32x32 block
                # transposes (keeps the tensor engine on expert matmuls)
                for j in range(4):
                    nc.vector.transpose(
                        g_sb[32 * j:32 * (j + 1), ti, :],
                        gexpT[:, ti * 128 + 32 * j:ti * 128 + 32 * (j + 1)])
            esum = stats.tile([128, 1], _F32, name="esum")
            nc.vector.tensor_reduce(esum[:, :], g_sb[:, ti, :E],
                                    axis=mybir.AxisListType.X,
                                    op=mybir.AluOpType.add)
            rec = stats.tile([128, 1], _F32, name="rec")
            nc.vector.reciprocal(rec[:, :], esum[:, :])
            nc.vector.tensor_scalar_mul(gn_sb[:, ti, :], g_sb[:, ti, :E],
                                        rec[:, :])
            if with_bias:
                # normalized gT for the bias-mix matmul
                tp2 = psum.tile([E, 128], _F32, name=f"tp2_{ti}", tag="ps")
                nc.tensor.transpose(tp2[:, :], gn_sb[:, ti, :], id_sb[:, :])
                nc.vector.tensor_copy(gtn[:, tsl], tp2[:, :])

        # --- experts ---
        stg_tiles = {}

        def final_piece(ti, lo, hi, ps):
            # last expert: stg-piece = ps * g + acc (bf16), optional bias
            # mix. Pieces accumulate into a per-ti staging tile.
            g_col = gn_sb[:, ti, E - 1:E]
            acc_sl = acc[:, ti, lo:hi]
            if ti not in stg_tiles:
                stg_tiles[ti] = stage_pool.tile([128, P], _BF16,
                                                name=f"stg{ti}", tag="stg")
            stg = stg_tiles[ti]
            if with_bias:
                ps_b = psum.tile([128, hi - lo], _F32,
                                 name=f"psb{ti}_{lo}", tag="ps")
                nc.tensor.matmul(
                    ps_b[:, :], gtn[:, ti * 128:(ti + 1) * 128],
                    eb_sb[:, lo:hi], start=True, stop=True)
                t1 = stage_pool.tile([128, hi - lo], _F32, name="t1",
                                     tag="t1")
                nc.vector.scalar_tensor_tensor(
                    t1[:, :], ps[:, :hi - lo], g_col, acc_sl,
                    op0=mybir.AluOpType.mult, op1=mybir.AluOpType.add)
                nc.vector.tensor_add(stg[:, lo:hi], t1[:, :], ps_b[:, :])
            else:
                nc.vector.scalar_tensor_tensor(
                    stg[:, lo:hi], ps[:, :hi - lo], g_col, acc_sl,
                    op0=mybir.AluOpType.mult, op1=mybir.AluOpType.add)

        def epilogue(e, ti, pc, ps):
            g_col = gn_sb[:, ti, e:e + 1]
            acc_sl = acc[:, ti, pc * PCHUNK:(pc + 1) * PCHUNK]
            if e == 0:
                # acc = ps * g on the otherwise-idle ACT engine
                # (per-partition scale AP); keeps the DVE free for the
                # gating transposes + later-expert stt chain.
                nc.scalar.activation(acc_sl, ps[:, :],
                                     mybir.ActivationFunctionType.Copy,
                                     scale=g_col)
            elif e < E - 1:
                nc.vector.scalar_tensor_tensor(
                    acc_sl, ps[:, :], g_col, acc_sl,
                    op0=mybir.AluOpType.mult, op1=mybir.AluOpType.add)
            else:
                final_piece(ti, pc * PCHUNK, (pc + 1) * PCHUNK, ps)
                if pc == PC - 1:
                    # whole token tile staged: one 256KB store
                    eng = nc.sync if ti % 2 == 0 else nc.scalar
                    eng.dma_start(out_t[:, ti, :], stg_tiles[ti][:, :])

        # gating transposes + phase-A epilogues (program order keeps
        # psum-pool rotation deadlock-free: transposes reuse the lgT
        # banks first, then epilogues free the ps_grp banks).
        for ti in range(TT):
            gate_tile(ti)
            if ti < TA:
                epilogue(0, ti, 0, ps_grp[ti])

        # expert-0 remainder: (ti TA..TT-1, pc0) first — they only need
        # the long-resident pc0 half, buying time for the merged w0 pc1
        # DMAs to land — then all pc1 groups. d-inner (xt resident).
        rem_groups = [(ti, 0) for ti in range(TA, TT)]
        rem_groups += [(ti, 1) for ti in range(TT)]
        for ti, pc in rem_groups:
            ps = psum.tile([128, PCHUNK], _F32, name=f"ps0_{ti}_{pc}",
                           tag="ps")
            for di in range(DT):
                nc.tensor.matmul(
                    ps[:, :], xt[:, di, ti * 128:(ti + 1) * 128],
                    w0[:, di, pc * PCHUNK:(pc + 1) * PCHUNK],
                    start=(di == 0), stop=(di == DT - 1))
            epilogue(0, ti, pc, ps)

        # experts 1..7, group-major: each output tile finishes its d-loop
        # early so the DVE epilogue chain spreads across the expert phase.
        for e in range(1, E):
            if e + 1 < E:
                issue_expert_dma(e + 1)
            wt = we_tiles.pop(e)
            for ti in range(TT):
                for pc in range(PC):
                    ps = psum.tile([128, PCHUNK], _F32,
                                   name=f"ps{e}_{ti}_{pc}", tag="ps")
                    for di in range(DT):
                        nc.tensor.matmul(
                            ps[:, :], xt[:, di, ti * 128:(ti + 1) * 128],
                            wt[:, di, pc * PCHUNK:(pc + 1) * PCHUNK],
                            start=(di == 0), stop=(di == DT - 1))
                    epilogue(e, ti, pc, ps)

    nc.compile()
    return nc


def _get_module(mode: str, with_bias: bool) -> bass.Bass:
    key = (mode, with_bias)
    if key not in _build_cache:
        _build_cache[key] = _build(mode, with_bias)
    return _build_cache[key]


_last_results = None


def _host_inputs(x, gate_w, gate_b, expert_w, expert_b, mode, with_bias):
    import ml_dtypes
    np_dt = ml_dtypes.bfloat16 if mode == "bf16" else np.float32

    x_flat = np.asarray(x, dtype=np.float32).reshape(TOK, D)
    gw_f = np.asarray(gate_w, np.float32)               # [D, E]
    gw_h = np.ascontiguousarray(
        gw_f.reshape(DT, 128, E).transpose(1, 0, 2).reshape(128, DT * E)
    ).astype(np_dt)
    ew_h = np.ascontiguousarray(np.asarray(expert_w, np.float32)).astype(np_dt)

    common = {"gate_w": gw_h, "expert_w": ew_h}
    if with_bias:
        common["ident"] = np.eye(128, dtype=np.float32)
        common["gate_b"] = np.asarray(gate_b, np.float32).reshape(E, 1)
        common["expert_b"] = np.asarray(expert_b, np.float32).astype(
            ml_dtypes.bfloat16)

    in_maps = []
    for c in range(N_CORES):
        shard = x_flat[c * TS:(c + 1) * TS]                  # [TS, D]
        xT_h = np.ascontiguousarray(shard.T).astype(np_dt)   # [D, TS]
        in_maps.append({"xT": xT_h, **common})
    return in_maps


def kernel(x, gate_w, gate_b, expert_w, expert_b):
    global _last_results
    mode = MM_DTYPE
    with_bias = bool(np.any(np.asarray(gate_b)) or np.any(np.asarray(expert_b)))
    nc = _get_module(mode, with_bias)
    in_maps = _host_inputs(x, gate_w, gate_b, expert_w, expert_b, mode,
                           with_bias)

    res = run_bass_kernel_spmd(nc, in_maps, core_ids=list(range(N_CORES)),
                               trace=TRACE)
    _last_results = res

    out = np.concatenate(
        [np.asarray(res.results[c]["out"], dtype=np.float32)
         for c in range(N_CORES)], axis=0)
    return out.reshape(B, T, P)


# revision 24
# speedup vs baseline: 1.0121x; 1.0121x over previous
"""MoE layer (dense experts) on 8 Trainium2 NeuronCores via Bass/Tile.

Problem (hardcoded shapes):
  x        [4, 2048, 1024] f32
  gate_w   [1024, 8] f32, gate_b [8] f32
  expert_w [8, 1024, 1024] f32, expert_b [8, 1024] f32
  out[b,t,p] = sum_e softmax(x @ gate_w + gate_b)[b,t,e]
               * (x @ expert_w[e] + expert_b[e])[b,t,p]

Sharding: data-parallel over tokens. 8192 tokens are split into 8 shards of
1024; every core gets the full gate/expert weights (replicated) and computes
its token shard end-to-end. No collectives.

Per-core kernel (x pre-transposed on host so the contraction dim is the
partition dim for both matmul operands):
  - gating logits computed TRANSPOSED: lgT[e, t] accumulates
    gw_d[128,8].T @ xT_d[128,512] over 8 d-tiles. exp via ACT, gates
    transposed to [tok, E] with DVE 32x32 stream transposes and
    normalized per 128-token tile.
  - expert e: psum[t128, p512] accumulates sum_d xT[d,t].T @ w_e[d,p] over
    8 d-tiles.
  - phase A is d-outer over (lgT0, ti0-3 pc0, lgT1, ti4-5 pc0): per-d-step
    HBM demand is xt tile (256KB) + w0 pc0 half (128KB) ~ 220GB/s, under
    the 358GB/s per-core HBM cap (interleaving both w0 halves would need
    ~300GB/s and stalls the PE during the ramp).
  - gate-weighted sum on DVE: acc = psum_e * g[:,e] + acc (one fused
    scalar_tensor_tensor per psum tile); the last expert's stt writes a
    per-ti bf16 staging tile, stored as one [128,1024] DMA per ti.
  - DMA count is kept low (~50 vs ~100): the end-of-program teardown and
    per-trigger engine cost (~700ns each) scale with it. Expert weights
    for e>=1 stream as 2x 1MB DMAs per expert, round-robin over the
    Sync/Scalar HWDGE + GpSimd SWDGE queues, issued one expert ahead.
  - PE warm-up: ~8 dummy matmuls over a memset tile bridge the HAM clock
    gate (PE at 1.2GHz until ~3.4us of sustained activity) while the
    first DMAs land.
  - biases: the harness inputs have gate_b = expert_b = 0, checked at
    runtime; the specialized no-bias module skips the bias-mix matmuls.
Matmul dtype: bf16 (default) or float32r/fp32 via MOE_MM_DTYPE.
"""

import os
from contextlib import ExitStack

import numpy as np

import concourse.bacc as bacc
import concourse.bass as bass
import concourse.mybir as mybir
import concourse.tile as tile
from concourse.bass_utils import run_bass_kernel_spmd

B, T, D, E, P = 4, 2048, 1024, 8, 1024
N_CORES = 8
TOK = B * T                # 8192 tokens
TS = TOK // N_CORES        # 1024 tokens per core
DT = D // 128              # 8 contraction tiles
TT = TS // 128             # 8 token tiles per core
PCHUNK = 512               # psum bank free size (f32)
PC = P // PCHUNK           # 2 p-chunks
TA = 6                     # token tiles in phase-A (pc0) d-outer phase
                           # (TA + 2 logit banks = 8 psum banks)
TCH = TS // PCHUNK         # 2 token chunks for the gating matmul
HALF = TS // 2             # xT d-tile DMA half (per-queue split)

_F32 = mybir.dt.float32
_BF16 = mybir.dt.bfloat16

MM_DTYPE = os.environ.get("MOE_MM_DTYPE", "bf16")
TRACE = os.environ.get("MOE_TRACE", "0") == "1"  # test.py sets this for profiling

_mm_dt = {
    "fp32r": mybir.dt.float32r,
    "bf16": mybir.dt.bfloat16,
    "fp32": mybir.dt.float32,
}

_build_cache = {}


def _build(mode: str, with_bias: bool) -> bass.Bass:
    mm = _mm_dt[mode]
    nc = bacc.Bacc("TRN2", target_bir_lowering=False, debug=False,
                   num_devices=N_CORES)

    xT = nc.dram_tensor("xT", [D, TS], mm, kind="ExternalInput").ap()
    # gate_w pre-arranged on host to [128, DT*E] (dp-major) for one
    # contiguous DMA
    gw = nc.dram_tensor("gate_w", [128, DT * E], mm, kind="ExternalInput").ap()
    ew = nc.dram_tensor("expert_w", [E, D, P], mm, kind="ExternalInput").ap()
    if with_bias:
        ident = nc.dram_tensor("ident", [128, 128], _F32,
                               kind="ExternalInput").ap()
        gb = nc.dram_tensor("gate_b", [E, 1], _F32, kind="ExternalInput").ap()
        eb = nc.dram_tensor("expert_b", [E, P], _BF16, kind="ExternalInput").ap()
    out = nc.dram_tensor("out", [TS, P], _BF16, kind="ExternalOutput").ap()

    out_t = out.rearrange("(tt tp) p -> tp tt p", tp=128)
    xT_t = xT.rearrange("(dt dp) t -> dp dt t", dp=128)
    # expert_w as [dp, e, dt, p] so a [128, ndt, P] slab is one DMA
    ew_t = ew.rearrange("e (dt dp) p -> dp e dt p", dp=128)

    with tile.TileContext(nc) as tc, ExitStack() as ctx:
        consts = ctx.enter_context(tc.tile_pool(name="consts", bufs=1))
        w0_pool = ctx.enter_context(tc.tile_pool(name="w0", bufs=1))
        we_pool = ctx.enter_context(tc.tile_pool(name="we", bufs=3))
        stage_pool = ctx.enter_context(tc.tile_pool(name="stage", bufs=3))
        stats = ctx.enter_context(tc.tile_pool(name="stats", bufs=4))
        psum = ctx.enter_context(tc.tile_pool(name="psum", bufs=8, space="PSUM"))

        # PE warm-up: the HAM clock gate keeps the PE at 1.2GHz until it
        # has seen ~3.4us of sustained matmul activity. The first real
        # matmul can't start before ~8us (framework preamble + DMA
        # latency), so burn the wait on dummy matmuls to enter warm.
        warm = consts.tile([128, PCHUNK], mm, name="warm", tag="warm")
        nc.vector.memset(warm[:, :], 0.0)
        wps = psum.tile([128, PCHUNK], _F32, name="warm_ps", tag="ps")
        for i in range(3):
            nc.tensor.matmul(wps[:, :], warm[:, :128], warm[:, :],
                             start=True, stop=True)

        # Ramp: gw first (first matmul needs it), then per-d-tile pieces in
        # phase-A consumption order: xt halves on the two HWDGE queues,
        # w0 pc0 half-tiles on the GpSimd SWDGE queue. The w0 pc1 halves
        # and expert-1 weights follow as merged DMAs.
        # Ramp queue plan: the three queues deliver ~1 piece per ~1.2us
        # each during the ramp (trigger + transfer + HBM-receipt bound),
        # so the first pieces are spread across ALL queues: gw leads Sync
        # (the Scalar queue runs ~1.4us behind — the framework's
        # ACT_TABLE_LOAD precedes its first trigger — so gw must not sit
        # there), xt h0 follows on Sync, xt h1 on Scalar, and the whole
        # w0 slab has the GpSimd SWDGE queue to itself.
        gw_sb = consts.tile([128, DT, E], mm, name="gw_sb", tag="gw_sb")
        nc.sync.dma_start(gw_sb[:, :, :],
                          gw.rearrange("dp (dt e) -> dp dt e", e=E))
        xt = consts.tile([128, DT, TS], mm, name="xt", tag="xt")
        w0 = consts.tile([128, DT, P], mm, name="w0", tag="w0")
        for di in range(DT):
            nc.sync.dma_start(xt[:, di, :HALF], xT_t[:, di, :HALF])
            nc.scalar.dma_start(xt[:, di, HALF:], xT_t[:, di, HALF:])
            nc.gpsimd.dma_start(w0[:, di, :PCHUNK], ew_t[:, 0, di, :PCHUNK])
        # w0 pc1 halves, merged: [128, 4, 512] each
        nc.gpsimd.dma_start(w0[:, 0:4, PCHUNK:], ew_t[:, 0, 0:4, PCHUNK:])
        nc.gpsimd.dma_start(w0[:, 4:8, PCHUNK:], ew_t[:, 0, 4:8, PCHUNK:])
        if with_bias:
            id_sb = consts.tile([128, 128], _F32, name="id_sb", tag="id_sb")
            nc.sync.dma_start(id_sb[:, :], ident)
            gb_sb = consts.tile([E, 1], _F32, name="gb_sb", tag="gb_sb")
            nc.sync.dma_start(gb_sb[:, :], gb)
            eb_sb = consts.tile([E, P], _BF16, name="eb_sb", tag="eb_sb")
            nc.sync.dma_start(eb_sb[:, :], eb)

        gn_sb = consts.tile([128, TT, E], _F32, name="gn_sb", tag="gn_sb")  # normalized
        if with_bias:
            gexpT = consts.tile([E, TS], _F32, name="gexpT", tag="gexpT")
            g_sb = consts.tile([128, TT, E], _F32, name="g_sb", tag="g_sb")
        else:
            # gexpT padded to 32 partitions so the DVE 32x32 stream
            # transpose can produce the [tok, E] gate layout without
            # spending tensor-engine time; rows E..31 stay zero
            gexpT = consts.tile([32, TS], _F32, name="gexpT", tag="gexpT")
            nc.vector.memset(gexpT[:, :], 0.0)
            g_sb = consts.tile([128, TT, 32], _F32, name="g_sb", tag="g_sb")
            junk = consts.tile([E, TCH], _F32, name="junk", tag="junk")
        if with_bias:
            gtn = consts.tile([E, TS], _BF16, name="gtn", tag="gtn")  # normalized gT
        acc = consts.tile([128, TT, P], _F32, name="acc", tag="acc")

        # expert e>=1 weight tiles: one [128, DT, P] slab per expert,
        # filled by two 1MB DMAs; we_pool bufs=3 gives ~2 experts of
        # prefetch lead. Queues split over the two HWDGE rings.
        we_tiles = {}

        _eq = [(nc.sync, nc.scalar), (nc.gpsimd, nc.sync),
               (nc.scalar, nc.gpsimd)]

        def issue_expert_dma(e):
            w_tile = we_pool.tile([128, DT, P], mm, name=f"we{e}", tag="we")
            qa, qb = _eq[(e - 1) % 3]
            qa.dma_start(w_tile[:, 0:4, :], ew_t[:, e, 0:4, :])
            qb.dma_start(w_tile[:, 4:8, :], ew_t[:, e, 4:8, :])
            we_tiles[e] = w_tile

        issue_expert_dma(1)

        # --- phase A: gating logits (transposed) + expert-0 pc0 for token
        # tiles 0..TA-1, d-outer so compute starts as each piece lands.
        lgT = [psum.tile([E, PCHUNK], _F32, name=f"lgT{tch}", tag="ps")
               for tch in range(TCH)]
        ps_grp = {}
        for ti in range(TA):
            ps_grp[ti] = psum.tile([128, PCHUNK], _F32,
                                   name=f"ps0_{ti}_0", tag="ps")
        # Per d-step order: lgT0 + ti0-3 need only the Sync-queue x half
        # (first to arrive: the Scalar queue's x halves run ~1.3us later
        # behind the ACT table load); lgT1 + ti4-5 need the Scalar half.
        # w0 demand is the pc0 half only (full-w0 interleave would need
        # ~300GB/s and stall the ramp).
        for di in range(DT):
            # step 0 runs both gating matmuls first: they depend only on
            # the fast HWDGE pieces and bridge the jittery SWDGE w0-d0
            # arrival; later steps put lgT1 after ti0-3 so the Scalar-
            # queue x half is never on the critical path.
            lg1_pos = 0 if di == 0 else 4
            nc.tensor.matmul(lgT[0][:, :], gw_sb[:, di, :],
                             xt[:, di, 0:PCHUNK],
                             start=(di == 0), stop=(di == DT - 1))
            if lg1_pos == 0:
                nc.tensor.matmul(lgT[1][:, :], gw_sb[:, di, :],
                                 xt[:, di, PCHUNK:2 * PCHUNK],
                                 start=(di == 0), stop=False)
            for ti in range(4):
                nc.tensor.matmul(
                    ps_grp[ti][:, :],
                    xt[:, di, ti * 128:(ti + 1) * 128],
                    w0[:, di, 0:PCHUNK],
                    start=(di == 0), stop=(di == DT - 1))
            if lg1_pos != 0:
                nc.tensor.matmul(lgT[1][:, :], gw_sb[:, di, :],
                                 xt[:, di, PCHUNK:2 * PCHUNK],
                                 start=False, stop=(di == DT - 1))
            for ti in range(4, TA):
                nc.tensor.matmul(
                    ps_grp[ti][:, :],
                    xt[:, di, ti * 128:(ti + 1) * 128],
                    w0[:, di, 0:PCHUNK],
                    start=(di == 0), stop=(di == DT - 1))

        # --- gating epilogue: exp (no max-sub), DVE transpose per token
        # tile, normalize in [tok, E] layout.
        for tch in range(TCH):
            sl = slice(tch * PCHUNK, (tch + 1) * PCHUNK)
            nc.scalar.activation(gexpT[:E, sl], lgT[tch][:, :],
                                 mybir.ActivationFunctionType.Exp,
                                 bias=gb_sb[:, :] if with_bias else 0.0,
                                 scale=1.0)
            if not with_bias:
                # ordering crutch: a DVE read of the exp output ahead of
                # the stream transposes in the DVE queue guarantees the
                # ACT->DVE dependency even if InstStreamTranspose inputs
                # aren't tracked across engines
                nc.vector.tensor_copy(junk[:, tch:tch + 1],
                                      gexpT[:E, tch * PCHUNK:
                                            tch * PCHUNK + 1])

        def gate_tile(ti):
            tsl = slice(ti * 128, (ti + 1) * 128)
            if with_bias:
                tp = psum.tile([128, E], _F32, name=f"tp{ti}", tag="ps")
                nc.tensor.transpose(tp[:, :], gexpT[:, tsl], id_sb[:E, :E])
                nc.vector.tensor_copy(g_sb[:, ti, :], tp[:, :])
            else:
                # [32-pad, 128] -> [128, 32-pad] via four DVE 32x32 block
                # transposes (keeps the tensor engine on expert matmuls)
                for j in range(4):
                    nc.vector.transpose(
                        g_sb[32 * j:32 * (j + 1), ti, :],
                        gexpT[:, ti * 128 + 32 * j:ti * 128 + 32 * (j + 1)])
            esum = stats.tile([128, 1], _F32, name="esum")
            nc.vector.tensor_reduce(esum[:, :], g_sb[:, ti, :E],
                                    axis=mybir.AxisListType.X,
                                    op=mybir.AluOpType.add)
            rec = stats.tile([128, 1], _F32, name="rec")
            nc.vector.reciprocal(rec[:, :], esum[:, :])
            nc.vector.tensor_scalar_mul(gn_sb[:, ti, :], g_sb[:, ti, :E],
                                        rec[:, :])
            if with_bias:
                # normalized gT for the bias-mix matmul
                tp2 = psum.tile([E, 128], _F32, name=f"tp2_{ti}", tag="ps")
                nc.tensor.transpose(tp2[:, :], gn_sb[:, ti, :], id_sb[:, :])
                nc.vector.tensor_copy(gtn[:, tsl], tp2[:, :])

        # --- experts ---
        stg_tiles = {}

        def final_piece(ti, lo, hi, ps):
            # last expert: stg-piece = ps * g + acc (bf16), optional bias
            # mix. Pieces accumulate into a per-ti staging tile.
            g_col = gn_sb[:, ti, E - 1:E]
            acc_sl = acc[:, ti, lo:hi]
            if ti not in stg_tiles:
                stg_tiles[ti] = stage_pool.tile([128, P], _BF16,
                                                name=f"stg{ti}", tag="stg")
            stg = stg_tiles[ti]
            if with_bias:
                ps_b = psum.tile([128, hi - lo], _F32,
                                 name=f"psb{ti}_{lo}", tag="ps")
                nc.tensor.matmul(
                    ps_b[:, :], gtn[:, ti * 128:(ti + 1) * 128],
                    eb_sb[:, lo:hi], start=True, stop=True)
                t1 = stage_pool.tile([128, hi - lo], _F32, name="t1",
                                     tag="t1")
                nc.vector.scalar_tensor_tensor(
                    t1[:, :], ps[:, :hi - lo], g_col, acc_sl,
                    op0=mybir.AluOpType.mult, op1=mybir.AluOpType.add)
                nc.vector.tensor_add(stg[:, lo:hi], t1[:, :], ps_b[:, :])
            else:
                nc.vector.scalar_tensor_tensor(
                    stg[:, lo:hi], ps[:, :hi - lo], g_col, acc_sl,
                    op0=mybir.AluOpType.mult, op1=mybir.AluOpType.add)

        def epilogue(e, ti, pc, ps):
            g_col = gn_sb[:, ti, e:e + 1]
            acc_sl = acc[:, ti, pc * PCHUNK:(pc + 1) * PCHUNK]
            if e == 0:
                # acc = ps * g on the otherwise-idle ACT engine
                # (per-partition scale AP); keeps the DVE free for the
                # gating transposes + later-expert stt chain.
                nc.scalar.activation(acc_sl, ps[:, :],
                                     mybir.ActivationFunctionType.Copy,
                                     scale=g_col)
            elif e < E - 1:
                nc.vector.scalar_tensor_tensor(
                    acc_sl, ps[:, :], g_col, acc_sl,
                    op0=mybir.AluOpType.mult, op1=mybir.AluOpType.add)
            else:
                final_piece(ti, pc * PCHUNK, (pc + 1) * PCHUNK, ps)
                if pc == PC - 1:
                    # whole token tile staged: one 256KB store
                    eng = nc.sync if ti % 2 == 0 else nc.scalar
                    eng.dma_start(out_t[:, ti, :], stg_tiles[ti][:, :])

        # gating transposes + phase-A epilogues (program order keeps
        # psum-pool rotation deadlock-free: transposes reuse the lgT
        # banks first, then epilogues free the ps_grp banks).
        for ti in range(TT):
            gate_tile(ti)
            if ti < TA:
                epilogue(0, ti, 0, ps_grp[ti])

        # expert-0 remainder: (ti TA..TT-1, pc0) first — they only need
        # the long-resident pc0 half, buying time for the merged w0 pc1
        # DMAs to land — then all pc1 groups. d-inner (xt resident).
        rem_groups = [(ti, 0) for ti in range(TA, TT)]
        rem_groups += [(ti, 1) for ti in range(TT)]
        for ti, pc in rem_groups:
            ps = psum.tile([128, PCHUNK], _F32, name=f"ps0_{ti}_{pc}",
                           tag="ps")
            for di in range(DT):
                nc.tensor.matmul(
                    ps[:, :], xt[:, di, ti * 128:(ti + 1) * 128],
                    w0[:, di, pc * PCHUNK:(pc + 1) * PCHUNK],
                    start=(di == 0), stop=(di == DT - 1))
            epilogue(0, ti, pc, ps)

        # experts 1..7, group-major: each output tile finishes its d-loop
        # early so the DVE epilogue chain spreads across the expert phase.
        for e in range(1, E):
            if e + 1 < E:
                issue_expert_dma(e + 1)
            wt = we_tiles.pop(e)
            for ti in range(TT):
                for pc in range(PC):
                    ps = psum.tile([128, PCHUNK], _F32,
                                   name=f"ps{e}_{ti}_{pc}", tag="ps")
                    for di in range(DT):
                        nc.tensor.matmul(
                            ps[:, :], xt[:, di, ti * 128:(ti + 1) * 128],
                            wt[:, di, pc * PCHUNK:(pc + 1) * PCHUNK],
                            start=(di == 0), stop=(di == DT - 1))
                    epilogue(e, ti, pc, ps)

    nc.compile()
    return nc


def _get_module(mode: str, with_bias: bool) -> bass.Bass:
    key = (mode, with_bias)
    if key not in _build_cache:
        _build_cache[key] = _build(mode, with_bias)
    return _build_cache[key]


_last_results = None


def _host_inputs(x, gate_w, gate_b, expert_w, expert_b, mode, with_bias):
    import ml_dtypes
    np_dt = ml_dtypes.bfloat16 if mode == "bf16" else np.float32

    x_flat = np.asarray(x, dtype=np.float32).reshape(TOK, D)
    gw_f = np.asarray(gate_w, np.float32)               # [D, E]
    gw_h = np.ascontiguousarray(
        gw_f.reshape(DT, 128, E).transpose(1, 0, 2).reshape(128, DT * E)
    ).astype(np_dt)
    ew_h = np.ascontiguousarray(np.asarray(expert_w, np.float32)).astype(np_dt)

    common = {"gate_w": gw_h, "expert_w": ew_h}
    if with_bias:
        common["ident"] = np.eye(128, dtype=np.float32)
        common["gate_b"] = np.asarray(gate_b, np.float32).reshape(E, 1)
        common["expert_b"] = np.asarray(expert_b, np.float32).astype(
            ml_dtypes.bfloat16)

    in_maps = []
    for c in range(N_CORES):
        shard = x_flat[c * TS:(c + 1) * TS]                  # [TS, D]
        xT_h = np.ascontiguousarray(shard.T).astype(np_dt)   # [D, TS]
        in_maps.append({"xT": xT_h, **common})
    return in_maps


def kernel(x, gate_w, gate_b, expert_w, expert_b):
    global _last_results
    mode = MM_DTYPE
    with_bias = bool(np.any(np.asarray(gate_b)) or np.any(np.asarray(expert_b)))
    nc = _get_module(mode, with_bias)
    in_maps = _host_inputs(x, gate_w, gate_b, expert_w, expert_b, mode,
                           with_bias)

    res = run_bass_kernel_spmd(nc, in_maps, core_ids=list(range(N_CORES)),
                               trace=TRACE)
    _last_results = res

    out = np.concatenate(
        [np.asarray(res.results[c]["out"], dtype=np.float32)
         for c in range(N_CORES)], axis=0)
    return out.reshape(B, T, P)


# revision 27
# speedup vs baseline: 1.0156x; 1.0035x over previous
"""MoE layer (dense experts) on 8 Trainium2 NeuronCores via Bass/Tile.

Problem (hardcoded shapes):
  x        [4, 2048, 1024] f32
  gate_w   [1024, 8] f32, gate_b [8] f32
  expert_w [8, 1024, 1024] f32, expert_b [8, 1024] f32
  out[b,t,p] = sum_e softmax(x @ gate_w + gate_b)[b,t,e]
               * (x @ expert_w[e] + expert_b[e])[b,t,p]

Sharding: data-parallel over tokens. 8192 tokens are split into 8 shards of
1024; every core gets the full gate/expert weights (replicated) and computes
its token shard end-to-end. No collectives.

Per-core kernel (x pre-transposed on host so the contraction dim is the
partition dim for both matmul operands):
  - gating logits computed TRANSPOSED: lgT[e, t] accumulates
    gw_d[128,8].T @ xT_d[128,512] over 8 d-tiles (stationary is the tiny
    8-col gate weight -> negligible LDWEIGHTS, 16 N=512 matmuls total).
    exp via ACT (per-partition bias gb if nonzero; no max subtraction:
    logits are ~N(0,1), exp is safe in f32). Gates are transposed to
    [tok, E] with DVE 32x32 stream transposes (a DVE read of the exp
    output first guarantees the cross-engine ACT->DVE ordering) and
    normalized per 128-token tile (reduce_add + reciprocal + mul).
    The tensor engine spends nothing on gating beyond the 16 logit
    matmuls; it is warmed up through the HAM clock gate by dummy
    matmuls while the first DMAs land.
  - expert e: psum[t128, p512] accumulates sum_d xT[d,t].T @ w_e[d,p] over
    8 d-tiles. Expert 0 runs d-outer over 3 token tiles (6 psum banks; the
    other 2 hold lgT) so compute starts as soon as the first xT/w0 d-tiles
    land; everything else is group-major so the DVE epilogue chain spreads.
  - gate-weighted sum on DVE: acc = psum_e * g[:,e] + acc (one fused
    scalar_tensor_tensor per psum tile); the last expert's stt writes the
    bf16 output staging tile directly.
  - biases: the harness's inputs have gate_b = expert_b = 0, checked at
    runtime; the specialized no-bias module skips the bias-mix matmuls.
    A general with-bias module (gate-weighted expert_b via a K=8 matmul
    with the normalized transposed gates) is built only if needed.
  - DMA triggers cost ~650ns and each issuing queue sustains ~90GB/s, so
    the ramp is spread over three queues: xT halves on Sync+Scalar HWDGE,
    expert weights on GpSimd SWDGE.
Matmul dtype: bf16 (default) or float32r/fp32 via MOE_MM_DTYPE.
"""

import os
from contextlib import ExitStack

import numpy as np

import concourse.bacc as bacc
import concourse.bass as bass
import concourse.mybir as mybir
import concourse.tile as tile
from concourse.bass_utils import run_bass_kernel_spmd

B, T, D, E, P = 4, 2048, 1024, 8, 1024
N_CORES = 8
TOK = B * T                # 8192 tokens
TS = TOK // N_CORES        # 1024 tokens per core
DT = D // 128              # 8 contraction tiles
TT = TS // 128             # 8 token tiles per core
PCHUNK = 512               # psum bank free size (f32)
PC = P // PCHUNK           # 2 p-chunks
TH = 3                     # token tiles in expert-0 d-outer phase
                           # (TH*PC + 2 logit banks = 8 psum banks)
TCH = TS // PCHUNK         # 2 token chunks for the gating matmul
HALF = TS // 2             # xT d-tile DMA half (per-queue split)

_F32 = mybir.dt.float32
_BF16 = mybir.dt.bfloat16

MM_DTYPE = os.environ.get("MOE_MM_DTYPE", "bf16")
TRACE = os.environ.get("MOE_TRACE", "0") == "1"  # test.py sets this for profiling

_mm_dt = {
    "fp32r": mybir.dt.float32r,
    "bf16": mybir.dt.bfloat16,
    "fp32": mybir.dt.float32,
}

_build_cache = {}


def _build(mode: str, with_bias: bool) -> bass.Bass:
    mm = _mm_dt[mode]
    nc = bacc.Bacc("TRN2", target_bir_lowering=False, debug=False,
                   num_devices=N_CORES)

    xT = nc.dram_tensor("xT", [D, TS], mm, kind="ExternalInput").ap()
    # gate_w pre-arranged on host to [128, DT*E] (dp-major) for one
    # contiguous DMA
    gw = nc.dram_tensor("gate_w", [128, DT * E], mm, kind="ExternalInput").ap()
    ew = nc.dram_tensor("expert_w", [E, D, P], mm, kind="ExternalInput").ap()
    if with_bias:
        ident = nc.dram_tensor("ident", [128, 128], _F32,
                               kind="ExternalInput").ap()
        gb = nc.dram_tensor("gate_b", [E, 1], _F32, kind="ExternalInput").ap()
        eb = nc.dram_tensor("expert_b", [E, P], _BF16, kind="ExternalInput").ap()
    out = nc.dram_tensor("out", [TS, P], _BF16, kind="ExternalOutput").ap()

    out_t = out.rearrange("(tt tp) p -> tp tt p", tp=128)
    xT_t = xT.rearrange("(dt dp) t -> dp dt t", dp=128)

    with tile.TileContext(nc) as tc, ExitStack() as ctx:
        consts = ctx.enter_context(tc.tile_pool(name="consts", bufs=1))
        w_pool = ctx.enter_context(tc.tile_pool(name="w", bufs=22))
        stage_pool = ctx.enter_context(tc.tile_pool(name="stage", bufs=6))
        stats = ctx.enter_context(tc.tile_pool(name="stats", bufs=4))
        psum = ctx.enter_context(tc.tile_pool(name="psum", bufs=8, space="PSUM"))

        # PE warm-up: the HAM clock gate keeps the PE at 1.2GHz until it
        # has seen ~3.4us of sustained matmul activity. The first real
        # matmul can't start before ~10us (DMA ramp), so burn the wait on
        # dummy matmuls over a memset tile to enter the kernel warm.
        warm = consts.tile([128, PCHUNK], mm, name="warm")
        nc.vector.memset(warm[:, :], 0.0)
        wps = psum.tile([128, PCHUNK], _F32, name="warm_ps", tag="ps")
        for i in range(8):
            nc.tensor.matmul(wps[:, :], warm[:, :128], warm[:, :],
                             start=True, stop=True)

        # Ramp: gw first (first matmul needs it), then xT d-tile halves
        # alternating over the two HWDGE queues; expert-0 weights stream
        # on the GpSimd SWDGE queue in parallel (first d-tile split so
        # the expert-0 pipeline starts earlier).
        gw_sb = consts.tile([128, DT, E], mm, name="gw_sb")
        nc.scalar.dma_start(gw_sb[:, :, :],
                            gw.rearrange("dp (dt e) -> dp dt e", e=E))
        xt = consts.tile([128, DT, TS], mm, name="xt")
        w0 = []
        for di in range(DT):
            nc.sync.dma_start(xt[:, di, :HALF], xT_t[:, di, :HALF])
            nc.scalar.dma_start(xt[:, di, HALF:], xT_t[:, di, HALF:])
            w_tile = w_pool.tile([128, P], mm, name=f"wt0_{di}", tag="wt")
            if di < 3:
                # halves: phase A is pc-major, so the first 3 expert MMs
                # of a d-step need only the first 512 w columns -- finer
                # completion granularity starts them earlier (the last
                # byte still arrives at the same time)
                nc.gpsimd.dma_start(w_tile[:, :PCHUNK],
                                    ew[0, di * 128:(di + 1) * 128, :PCHUNK])
                nc.gpsimd.dma_start(w_tile[:, PCHUNK:],
                                    ew[0, di * 128:(di + 1) * 128, PCHUNK:])
            else:
                nc.gpsimd.dma_start(w_tile[:, :],
                                    ew[0, di * 128:(di + 1) * 128, :])
            w0.append(w_tile)
        if with_bias:
            id_sb = consts.tile([128, 128], _F32, name="id_sb")
            nc.sync.dma_start(id_sb[:, :], ident)
            gb_sb = consts.tile([E, 1], _F32, name="gb_sb")
            nc.sync.dma_start(gb_sb[:, :], gb)
            eb_sb = consts.tile([E, P], _BF16, name="eb_sb")
            nc.sync.dma_start(eb_sb[:, :], eb)

        gn_sb = consts.tile([128, TT, E], _F32, name="gn_sb")  # normalized
        if with_bias:
            gexpT = consts.tile([E, TS], _F32, name="gexpT")
            g_sb = consts.tile([128, TT, E], _F32, name="g_sb")
        else:
            # gexpT padded to 32 partitions so the DVE 32x32 stream
            # transpose can produce the [tok, E] gate layout without
            # spending tensor-engine time; rows E..31 stay zero
            gexpT = consts.tile([32, TS], _F32, name="gexpT")
            nc.vector.memset(gexpT[:, :], 0.0)
            g_sb = consts.tile([128, TT, 32], _F32, name="g_sb")
            junk = consts.tile([E, TCH], _F32, name="junk")
        if with_bias:
            gtn = consts.tile([E, TS], _BF16, name="gtn")  # normalized gT
        acc = consts.tile([128, TT, P], _F32, name="acc")

        # --- phase A: gating logits (transposed) + expert-0 first half,
        # d-outer so compute starts as soon as each xT/w0 d-tile lands.
        lgT = [psum.tile([E, PCHUNK], _F32, name=f"lgT{tch}", tag="ps")
               for tch in range(TCH)]
        ps_grp = {}
        for ti in range(TH):
            for pc in range(PC):
                ps_grp[ti, pc] = psum.tile([128, PCHUNK], _F32,
                                           name=f"ps0_{ti}_{pc}", tag="ps")
        for di in range(DT):
            # tch0 first (needs only the Sync-queue x half), then the
            # expert-0 groups pc-major (token tiles 0..TH-1 are inside x
            # half 0; pc=0 needs only the first w0 half), tch1 last so
            # the PE never stalls on the Scalar-queue x half.
            nc.tensor.matmul(lgT[0][:, :], gw_sb[:, di, :],
                             xt[:, di, 0:PCHUNK],
                             start=(di == 0), stop=(di == DT - 1))
            for pc in range(PC):
                for ti in range(TH):
                    nc.tensor.matmul(
                        ps_grp[ti, pc][:, :],
                        xt[:, di, ti * 128:(ti + 1) * 128],
                        w0[di][:, pc * PCHUNK:(pc + 1) * PCHUNK],
                        start=(di == 0), stop=(di == DT - 1))
            nc.tensor.matmul(lgT[1][:, :], gw_sb[:, di, :],
                             xt[:, di, PCHUNK:2 * PCHUNK],
                             start=(di == 0), stop=(di == DT - 1))

        # --- gating epilogue: exp (no max-sub), PE transpose per token
        # tile, normalize in [tok, E] layout.
        for tch in range(TCH):
            sl = slice(tch * PCHUNK, (tch + 1) * PCHUNK)
            nc.scalar.activation(gexpT[:E, sl], lgT[tch][:, :],
                                 mybir.ActivationFunctionType.Exp,
                                 bias=gb_sb[:, :] if with_bias else 0.0,
                                 scale=1.0)
            if not with_bias:
                # ordering crutch: a DVE read of the exp output ahead of
                # the stream transposes in the DVE queue guarantees the
                # ACT->DVE dependency even if InstStreamTranspose inputs
                # aren't tracked across engines
                nc.vector.tensor_copy(junk[:, tch:tch + 1],
                                      gexpT[:E, tch * PCHUNK:
                                            tch * PCHUNK + 1])

        def gate_tile(ti):
            tsl = slice(ti * 128, (ti + 1) * 128)
            if with_bias:
                tp = psum.tile([128, E], _F32, name=f"tp{ti}", tag="ps")
                nc.tensor.transpose(tp[:, :], gexpT[:, tsl], id_sb[:E, :E])
                nc.vector.tensor_copy(g_sb[:, ti, :], tp[:, :])
            else:
                # [32-pad, 128] -> [128, 32-pad] via four DVE 32x32 block
                # transposes (keeps the tensor engine on expert matmuls)
                for j in range(4):
                    nc.vector.transpose(
                        g_sb[32 * j:32 * (j + 1), ti, :],
                        gexpT[:, ti * 128 + 32 * j:ti * 128 + 32 * (j + 1)])
            esum = stats.tile([128, 1], _F32, name="esum")
            nc.vector.tensor_reduce(esum[:, :], g_sb[:, ti, :E],
                                    axis=mybir.AxisListType.X,
                                    op=mybir.AluOpType.add)
            rec = stats.tile([128, 1], _F32, name="rec")
            nc.vector.reciprocal(rec[:, :], esum[:, :])
            nc.vector.tensor_scalar_mul(gn_sb[:, ti, :], g_sb[:, ti, :E],
                                        rec[:, :])
            if with_bias:
                # normalized gT for the bias-mix matmul
                tp2 = psum.tile([E, 128], _F32, name=f"tp2_{ti}", tag="ps")
                nc.tensor.transpose(tp2[:, :], gn_sb[:, ti, :], id_sb[:, :])
                nc.vector.tensor_copy(gtn[:, tsl], tp2[:, :])

        # --- experts ---
        def store(ti, pc, j, h, stg):
            # alternate store queues so the out stream isn't serialized
            # on one ~90GB/s DMA queue during the last expert phase
            eng = nc.sync if (ti * PC + pc) % 2 == 0 else nc.scalar
            eng.dma_start(
                out_t[:, ti, pc * PCHUNK + j * h:pc * PCHUNK + (j + 1) * h],
                stg[:, :])

        def final_tile(ti, pc, ps, j=0, h=PCHUNK):
            # last expert: stg = ps * g + acc (bf16), optional bias mix,
            # then store.
            g_col = gn_sb[:, ti, E - 1:E]
            acc_sl = acc[:, ti, pc * PCHUNK + j * h:pc * PCHUNK + (j + 1) * h]
            if with_bias:
                ps_b = psum.tile([128, h], _F32, name=f"psb{ti}_{pc}_{j}",
                                 tag="ps")
                nc.tensor.matmul(
                    ps_b[:, :], gtn[:, ti * 128:(ti + 1) * 128],
                    eb_sb[:, pc * PCHUNK + j * h:pc * PCHUNK + (j + 1) * h],
                    start=True, stop=True)
                t1 = stage_pool.tile([128, h], _F32, name="t1")
                nc.vector.scalar_tensor_tensor(
                    t1[:, :], ps[:, :h], g_col, acc_sl,
                    op0=mybir.AluOpType.mult, op1=mybir.AluOpType.add)
                stg = stage_pool.tile([128, h], _BF16, name="stg")
                nc.vector.tensor_add(stg[:, :], t1[:, :], ps_b[:, :])
            else:
                stg = stage_pool.tile([128, h], _BF16, name="stg")
                nc.vector.scalar_tensor_tensor(
                    stg[:, :], ps[:, :h], g_col, acc_sl,
                    op0=mybir.AluOpType.mult, op1=mybir.AluOpType.add)
            if ti == TT - 1 and pc == PC - 1 and j == 1:
                # very last piece: store quarters on both queues so the
                # final transfer is as short as possible
                q = h // 2
                base = pc * PCHUNK + j * h
                nc.sync.dma_start(out_t[:, ti, base:base + q], stg[:, :q])
                nc.scalar.dma_start(out_t[:, ti, base + q:base + 2 * q],
                                    stg[:, q:])
                return
            store(ti, pc, j, h, stg)

        def epilogue(e, ti, pc, ps):
            g_col = gn_sb[:, ti, e:e + 1]
            acc_sl = acc[:, ti, pc * PCHUNK:(pc + 1) * PCHUNK]
            if e == 0:
                # acc = ps * g on the otherwise-idle ACT engine
                # (per-partition scale AP); keeps the DVE free for the
                # gating transposes + later-expert stt chain, whose
                # PSUM-bank drains gate the expert-1 matmul groups.
                nc.scalar.activation(acc_sl, ps[:, :],
                                     mybir.ActivationFunctionType.Copy,
                                     scale=g_col)
            elif e < E - 1:
                nc.vector.scalar_tensor_tensor(
                    acc_sl, ps[:, :], g_col, acc_sl,
                    op0=mybir.AluOpType.mult, op1=mybir.AluOpType.add)
            else:
                final_tile(ti, pc, ps)

        # gating transposes + expert-0 first-half epilogues (program order
        # keeps psum-pool rotation deadlock-free: transposes reuse the lgT
        # banks first, then epilogues free the ps_grp banks).
        for ti in range(TT):
            gate_tile(ti)
            if ti < TH:
                for pc in range(PC):
                    epilogue(0, ti, pc, ps_grp[ti, pc])

        # expert-0 second half, group-major
        for ti in range(TH, TT):
            for pc in range(PC):
                ps = psum.tile([128, PCHUNK], _F32,
                               name=f"ps0_{ti}_{pc}", tag="ps")
                for di in range(DT):
                    nc.tensor.matmul(
                        ps[:, :], xt[:, di, ti * 128:(ti + 1) * 128],
                        w0[di][:, pc * PCHUNK:(pc + 1) * PCHUNK],
                        start=(di == 0), stop=(di == DT - 1))
                epilogue(0, ti, pc, ps)

        # experts 1..7, group-major: each output tile finishes its d-loop
        # early so the DVE epilogue chain spreads across the expert phase.
        w_engines = [nc.gpsimd, nc.sync, nc.scalar]
        for e in range(1, E):
            wt = []
            for di in range(DT):
                w_tile = w_pool.tile([128, P], mm, name=f"wt{e}_{di}",
                                     tag="wt")
                # round-robin the weight stream over all three DMA queues
                # (Sync/Scalar are idle once the ramp is done)
                w_engines[di % 3].dma_start(
                    w_tile[:, :], ew[e, di * 128:(di + 1) * 128, :])
                wt.append(w_tile)
            for ti in range(TT):
                for pc in range(PC):
                    # (the last output tile is NOT split into halves: the
                    # post-matmul stt+store chain hides entirely under the
                    # framework's ~11us end-of-program teardown, so extra
                    # matmuls for tail pipelining are a pure loss)
                    ps = psum.tile([128, PCHUNK], _F32,
                                   name=f"ps{e}_{ti}_{pc}", tag="ps")
                    for di in range(DT):
                        nc.tensor.matmul(
                            ps[:, :], xt[:, di, ti * 128:(ti + 1) * 128],
                            wt[di][:, pc * PCHUNK:(pc + 1) * PCHUNK],
                            start=(di == 0), stop=(di == DT - 1))
                    epilogue(e, ti, pc, ps)

    nc.compile()
    return nc


def _get_module(mode: str, with_bias: bool) -> bass.Bass:
    key = (mode, with_bias)
    if key not in _build_cache:
        _build_cache[key] = _build(mode, with_bias)
    return _build_cache[key]


_last_results = None


def _host_inputs(x, gate_w, gate_b, expert_w, expert_b, mode, with_bias):
    import ml_dtypes
    np_dt = ml_dtypes.bfloat16 if mode == "bf16" else np.float32

    x_flat = np.asarray(x, dtype=np.float32).reshape(TOK, D)
    gw_f = np.asarray(gate_w, np.float32)               # [D, E]
    gw_h = np.ascontiguousarray(
        gw_f.reshape(DT, 128, E).transpose(1, 0, 2).reshape(128, DT * E)
    ).astype(np_dt)
    ew_h = np.ascontiguousarray(np.asarray(expert_w, np.float32)).astype(np_dt)

    common = {"gate_w": gw_h, "expert_w": ew_h}
    if with_bias:
        common["ident"] = np.eye(128, dtype=np.float32)
        common["gate_b"] = np.asarray(gate_b, np.float32).reshape(E, 1)
        common["expert_b"] = np.asarray(expert_b, np.float32).astype(
            ml_dtypes.bfloat16)

    in_maps = []
    for c in range(N_CORES):
        shard = x_flat[c * TS:(c + 1) * TS]                  # [TS, D]
        xT_h = np.ascontiguousarray(shard.T).astype(np_dt)   # [D, TS]
        in_maps.append({"xT": xT_h, **common})
    return in_maps


def kernel(x, gate_w, gate_b, expert_w, expert_b):
    global _last_results
    mode = MM_DTYPE
    with_bias = bool(np.any(np.asarray(gate_b)) or np.any(np.asarray(expert_b)))
    nc = _get_module(mode, with_bias)
    in_maps = _host_inputs(x, gate_w, gate_b, expert_w, expert_b, mode,
                           with_bias)

    res = run_bass_kernel_spmd(nc, in_maps, core_ids=list(range(N_CORES)),
                               trace=TRACE)
    _last_results = res

    out = np.concatenate(
        [np.asarray(res.results[c]["out"], dtype=np.float32)
         for c in range(N_CORES)], axis=0)
    return out.reshape(B, T, P)

